# revision 2
# baseline (speedup 1.0000x reference)
"""Trainium2 Bass kernel for nn_NDNRefinement (4-layer GNN message passing), v2.

Strategy (8 NeuronCores):
- Sort triples by s_idx on host; core c owns triples whose subject falls in
  [c*OS, (c+1)*OS). Subject-side pooled rows are written to a local `stage`
  buffer; object-side rows go to fixed-size per-destination buckets that are
  exchanged with one AllToAll per layer. Per-object MLPs are data-parallel
  over the object shard; the new object table is AllGathered between layers.
- v2 rework vs v1: one batched 512-row indirect gather (s+o rows together)
  and one batched 512-row indirect scatter per triple block; all index /
  schedule arrays preloaded into SBUF once; 1/count scaling folded into the
  producer-side activation scale; W1b bias added with a K=1 matmul so PSUM
  goes straight to the activation; pooling matmuls emit feature-major
  pooled tiles directly (no post-pool transposes).
"""

import numpy as np
import ml_dtypes

import concourse.bass as bass
import concourse.bacc as bacc
import concourse.tile as tile
from concourse import mybir
from concourse.bass_utils import run_bass_kernel_spmd
from concourse.masks import make_identity

BF16 = mybir.dt.bfloat16
F32 = mybir.dt.float32
I32 = mybir.dt.int32
P = 128
NC = 8
ALPHA = 0.2

# (din, h, dout) per layer
DIMS = [(64, 512, 128), (128, 512, 128), (128, 512, 128), (128, 128, 128)]


def _rup(x, m):
    return ((int(x) + m - 1) // m) * m


# ---------------------------------------------------------------------------
# Host preprocessing
# ---------------------------------------------------------------------------

def preprocess(inputs):
    obj_vecs = np.asarray(inputs["obj_vecs"], np.float32)
    pred_vecs = np.asarray(inputs["pred_vecs"], np.float32)
    pred_boxes = np.asarray(inputs["pred_boxes"], np.float32)
    s_idx = np.asarray(inputs["s_idx"], np.int32)
    o_idx = np.asarray(inputs["o_idx"], np.int32)

    O = obj_vecs.shape[0]
    assert O % NC == 0
    OS = O // NC
    OSP = _rup(OS, P)
    NT = OSP // P
    OG = NC * OSP

    def gmap(idx):
        return ((idx // OS) * OSP + (idx % OS)).astype(np.int64)

    order = np.argsort(s_idx, kind="stable")
    s_sorted = s_idx[order]
    o_sorted = o_idx[order]
    bnd = np.searchsorted(s_sorted, np.arange(NC + 1) * OS)
    counts_c = np.diff(bnd)
    T_PC = max(_rup(counts_c.max(), 512), 512)
    NB = T_PC // 512

    cnt = np.bincount(s_idx, minlength=O) + np.bincount(o_idx, minlength=O)
    inv_cnt = (1.0 / np.maximum(cnt, 1)).astype(np.float32)

    percore = []
    maxB = 0
    max_s_load = 0
    for c in range(NC):
        sl = slice(bnd[c], bnd[c + 1])
        idxs = order[sl]
        n = len(idxs)
        s_c = s_sorted[sl]
        o_c = o_sorted[sl]
        d_c = (o_c // OS).astype(np.int64)
        ordb = np.lexsort((o_c, d_c))
        rank = np.empty(n, np.int64)
        d_srt = d_c[ordb]
        first = np.searchsorted(d_srt, np.arange(NC))
        rank[ordb] = np.arange(n) - first[d_srt]
        bc = np.bincount(d_c, minlength=NC)
        maxB = max(maxB, int(bc.max()) if n else 0)
        sload = np.bincount((s_c - c * OS) // P, minlength=NT)
        max_s_load = max(max_s_load, int(sload.max()) if n else 0)
        percore.append(dict(idxs=idxs, n=n, s_c=s_c, o_c=o_c, d_c=d_c,
                            rank=rank, bc=bc))

    n_pad_max = max(int(T_PC - pc["n"]) for pc in percore)
    S_B = _rup(maxB + n_pad_max // NC + 2, P)

    o_tiles_per_core = []
    max_o_load = 0
    for c in range(NC):
        rows_all, locs_all = [], []
        for d in range(NC):
            pc = percore[d]
            m = pc["d_c"] == c
            rows_all.append(d * S_B + pc["rank"][m])
            locs_all.append(pc["o_c"][m] - c * OS)
        rows_all = np.concatenate(rows_all)
        locs_all = np.concatenate(locs_all)
        oload = np.bincount(locs_all // P, minlength=NT)
        max_o_load = max(max_o_load, int(oload.max()) if len(locs_all) else 0)
        o_tiles_per_core.append((rows_all, locs_all))

    PS = max(1, -(-int(max_s_load) // P))
    PO = max(1, -(-int(max_o_load) // P))

    cfg = dict(O=O, OS=OS, OSP=OSP, NT=NT, OG=OG, T_PC=T_PC, NB=NB,
               S_B=S_B, PS=PS, PO=PO)

    bf = ml_dtypes.bfloat16
    shared = {}
    shared["w_emb"] = np.asarray(inputs["W_emb"], np.float32).astype(bf)
    shared["b_emb"] = np.asarray(inputs["b_emb"], np.float32).reshape(-1, 1)
    for li, (din, h, dout) in enumerate(DIMS):
        b1b = np.asarray(inputs[f"b1b{li}"], np.float32)
        shared[f"w1a{li}"] = np.asarray(inputs[f"W1a{li}"], np.float32).astype(bf)
        shared[f"w1b{li}"] = np.asarray(inputs[f"W1b{li}"], np.float32).astype(bf)
        shared[f"w2a{li}"] = np.asarray(inputs[f"W2a{li}"], np.float32).astype(bf)
        shared[f"w2b{li}"] = np.asarray(inputs[f"W2b{li}"], np.float32).astype(bf)
        shared[f"b1a{li}"] = np.asarray(inputs[f"b1a{li}"], np.float32).reshape(-1, P).T.copy()
        shared[f"b1bp{li}"] = b1b[h:h + dout].reshape(-1, 1).copy()
        shared[f"b1bsr{li}"] = b1b[:h].astype(bf).reshape(1, h).copy()
        shared[f"b1bor{li}"] = b1b[h + dout:].astype(bf).reshape(1, h).copy()
        shared[f"b2a{li}"] = np.asarray(inputs[f"b2a{li}"], np.float32).reshape(-1, P).T.copy()
        shared[f"b2b{li}"] = np.asarray(inputs[f"b2b{li}"], np.float32).reshape(-1, 1).copy()
    shared["wbb"] = np.asarray(inputs["W_bb"], np.float32).astype(bf)
    shared["bbb"] = np.asarray(inputs["b_bb"], np.float32).reshape(-1, 1)

    x_full = np.concatenate([obj_vecs, pred_boxes], axis=1)

    in_maps = []
    for c in range(NC):
        pc = percore[c]
        idxs, n = pc["idxs"], pc["n"]
        m = {}
        xT = np.zeros((68, OSP), bf)
        xT[:, :OS] = x_full[c * OS:(c + 1) * OS].T.astype(bf)
        m["xt"] = xT
        pT = np.zeros((64, T_PC), bf)
        pT[:, :n] = pred_vecs[idxs].T.astype(bf)
        m["pred0"] = pT

        # interleaved gather rows: block j, cols j*8+g: g<4 subject rows,
        # g>=4 object rows (global padded table ids); pads -> row 0
        sg_ = np.zeros((T_PC,), np.int64)
        sg_[:n] = gmap(pc["s_c"])
        og_ = np.zeros((T_PC,), np.int64)
        og_[:n] = gmap(pc["o_c"])
        sgog = np.zeros((NB, 8, P), np.int32)
        sgog[:, 0:4, :] = sg_.reshape(NB, 4, P)
        sgog[:, 4:8, :] = og_.reshape(NB, 4, P)
        m["sgog"] = sgog.reshape(NB * 8, P).T.copy()

        # bucket scatter positions; pad entries spread over per-bucket headroom
        ob_ = np.empty((T_PC,), np.int64)
        ob_[:n] = (pc["d_c"] * S_B + pc["rank"])
        npad = T_PC - n
        if npad:
            i = np.arange(npad)
            d = i % NC
            slot = S_B - 1 - (i // NC)
            assert (slot >= pc["bc"][d]).all(), "trash slots collide with data"
            ob_[n:] = (d * S_B + slot)
        m["ob"] = ob_.astype(np.int32).reshape(NB * 4, P).T.copy()

        # producer-side 1/count scales (pads -> 0)
        iv_s = np.zeros((T_PC,), np.float32)
        iv_s[:n] = inv_cnt[pc["s_c"]]
        iv_o = np.zeros((T_PC,), np.float32)
        iv_o[:n] = inv_cnt[pc["o_c"]]
        m["invs"] = iv_s.reshape(NB * 4, P).T.copy()
        m["invo"] = iv_o.reshape(NB * 4, P).T.copy()

        # s-pool schedule: ids into stage rows, locs relative to tile
        s_loc = pc["s_c"] - c * OS
        sids = np.zeros((NT, PS, P), np.int32)
        slocs = np.full((NT, PS, P), -1, np.int32)
        tstart = np.searchsorted(s_loc, np.arange(NT + 1) * P)
        for t in range(NT):
            a, b = int(tstart[t]), int(tstart[t + 1])
            k = b - a
            fi = np.zeros((PS * P,), np.int32)
            fl = np.full((PS * P,), -1, np.int32)
            fi[:k] = np.arange(a, b, dtype=np.int32)
            fl[:k] = (s_loc[a:b] - t * P).astype(np.int32)
            sids[t] = fi.reshape(PS, P)
            slocs[t] = fl.reshape(PS, P)
        m["sids"] = sids.reshape(NT * PS, P).T.copy()
        m["slocs"] = slocs.reshape(NT * PS, P).T.copy()

        # o-pool schedule: ids into recv rows
        rows_all, locs_all = o_tiles_per_core[c]
        oids = np.zeros((NT, PO, P), np.int32)
        olocs = np.full((NT, PO, P), -1, np.int32)
        tsel = locs_all // P
        for t in range(NT):
            msk = tsel == t
            k = int(msk.sum())
            fi = np.zeros((PO * P,), np.int32)
            fl = np.full((PO * P,), -1, np.int32)
            fi[:k] = rows_all[msk]
            fl[:k] = (locs_all[msk] - t * P)
            oids[t] = fi.reshape(PO, P)
            olocs[t] = fl.reshape(PO, P)
        m["oids"] = oids.reshape(NT * PO, P).T.copy()
        m["olocs"] = olocs.reshape(NT * PO, P).T.copy()

        m.update(shared)
        in_maps.append(m)

    return cfg, in_maps


# ---------------------------------------------------------------------------
# Kernel builder
# ---------------------------------------------------------------------------

def build_kernel(cfg):
    OSP, NT, OG = cfg["OSP"], cfg["NT"], cfg["OG"]
    T_PC, NB, S_B = cfg["T_PC"], cfg["NB"], cfg["S_B"]
    PS, PO = cfg["PS"], cfg["PO"]

    nc = bacc.Bacc("TRN2", target_bir_lowering=False, debug=False,
                   num_devices=NC)

    # ---- parameters ----
    xt = nc.declare_dram_parameter("xt", [68, OSP], BF16, isOutput=False)
    pred0 = nc.declare_dram_parameter("pred0", [64, T_PC], BF16, isOutput=False)
    sgog = nc.declare_dram_parameter("sgog", [P, NB * 8], I32, isOutput=False)
    ob = nc.declare_dram_parameter("ob", [P, NB * 4], I32, isOutput=False)
    invs = nc.declare_dram_parameter("invs", [P, NB * 4], F32, isOutput=False)
    invo = nc.declare_dram_parameter("invo", [P, NB * 4], F32, isOutput=False)
    sids = nc.declare_dram_parameter("sids", [P, NT * PS], I32, isOutput=False)
    slocs = nc.declare_dram_parameter("slocs", [P, NT * PS], I32, isOutput=False)
    oids = nc.declare_dram_parameter("oids", [P, NT * PO], I32, isOutput=False)
    olocs = nc.declare_dram_parameter("olocs", [P, NT * PO], I32, isOutput=False)

    w_emb = nc.declare_dram_parameter("w_emb", [68, 64], BF16, isOutput=False)
    b_emb = nc.declare_dram_parameter("b_emb", [64, 1], F32, isOutput=False)
    wp = {}
    for li, (din, h, dout) in enumerate(DIMS):
        wp[f"w1a{li}"] = nc.declare_dram_parameter(f"w1a{li}", [3 * din, h], BF16, isOutput=False)
        wp[f"w1b{li}"] = nc.declare_dram_parameter(f"w1b{li}", [h, 2 * h + dout], BF16, isOutput=False)
        wp[f"w2a{li}"] = nc.declare_dram_parameter(f"w2a{li}", [h, h], BF16, isOutput=False)
        wp[f"w2b{li}"] = nc.declare_dram_parameter(f"w2b{li}", [h, dout], BF16, isOutput=False)
        wp[f"b1a{li}"] = nc.declare_dram_parameter(f"b1a{li}", [P, h // P], F32, isOutput=False)
        wp[f"b1bp{li}"] = nc.declare_dram_parameter(f"b1bp{li}", [dout, 1], F32, isOutput=False)
        wp[f"b1bsr{li}"] = nc.declare_dram_parameter(f"b1bsr{li}", [1, h], BF16, isOutput=False)
        wp[f"b1bor{li}"] = nc.declare_dram_parameter(f"b1bor{li}", [1, h], BF16, isOutput=False)
        wp[f"b2a{li}"] = nc.declare_dram_parameter(f"b2a{li}", [P, h // P], F32, isOutput=False)
        wp[f"b2b{li}"] = nc.declare_dram_parameter(f"b2b{li}", [dout, 1], F32, isOutput=False)
    wbb = nc.declare_dram_parameter("wbb", [P, 4], BF16, isOutput=False)
    bbb = nc.declare_dram_parameter("bbb", [4, 1], F32, isOutput=False)

    out = nc.declare_dram_parameter("out", [4, OSP], F32, isOutput=True)

    # ---- internal DRAM ----
    tabs = [nc.dram_tensor("tab0", [OG, 64], BF16, addr_space="Shared")]
    for li in range(1, 4):
        tabs.append(nc.dram_tensor(f"tab{li}", [OG, P], BF16, addr_space="Shared"))
    preds = [pred0]
    for li in range(1, 4):
        preds.append(nc.dram_tensor(f"pred{li}", [P, T_PC], BF16))
    stages, sends, recvs, agins = [], [], [], []
    for li, (din, h, dout) in enumerate(DIMS):
        stages.append(nc.dram_tensor(f"stage{li}", [T_PC, h], BF16))
        sends.append(nc.dram_tensor(f"send{li}", [NC * S_B, h], BF16))
        recvs.append(nc.dram_tensor(f"recv{li}", [NC * S_B, h], BF16))
    agins.append(nc.dram_tensor("agin_e", [OSP, 64], BF16))
    for li in range(3):
        agins.append(nc.dram_tensor(f"agin{li}", [OSP, P], BF16))

    PRELU = mybir.ActivationFunctionType.Prelu
    GRPS = [list(range(NC))]

    with tile.TileContext(nc) as tc:
        with tc.tile_pool(name="cst", bufs=1) as cst:
            # constants
            ident = cst.tile([P, P], F32)
            make_identity(nc, ident[:])
            ident_bf = cst.tile([P, P], BF16)
            nc.vector.tensor_copy(out=ident_bf[:], in_=ident[:])
            iota = cst.tile([P, P], I32)
            nc.gpsimd.iota(iota[:], pattern=[[1, P]], base=0, channel_multiplier=0)
            ones_row = cst.tile([1, P], BF16)
            nc.vector.memset(ones_row[:], 1.0)

            W = {}

            def load_w(name, src_ap, hh, ww, dt):
                t = cst.tile([hh, ww], dt, tag=name)
                nc.sync.dma_start(out=t[:], in_=src_ap)
                W[name] = t

            load_w("w_emb", w_emb[:, :], 68, 64, BF16)
            load_w("b_emb", b_emb[:, :], 64, 1, F32)
            load_w("wbb", wbb[:, :], P, 4, BF16)
            load_w("bbb", bbb[:, :], 4, 1, F32)
            # schedule arrays, resident for the whole kernel
            load_w("sgog", sgog[:, :], P, NB * 8, I32)
            load_w("ob", ob[:, :], P, NB * 4, I32)
            load_w("invs", invs[:, :], P, NB * 4, F32)
            load_w("invo", invo[:, :], P, NB * 4, F32)
            load_w("sids", sids[:, :], P, NT * PS, I32)
            load_w("slocs", slocs[:, :], P, NT * PS, I32)
            load_w("oids", oids[:, :], P, NT * PO, I32)
            load_w("olocs", olocs[:, :], P, NT * PO, I32)
            for li, (din, h, dout) in enumerate(DIMS):
                for ki in range(3):
                    load_w(f"w1a{li}_c{ki}", wp[f"w1a{li}"][ki * din:(ki + 1) * din, :],
                           din, h, BF16)
                for k in range(h // P):
                    load_w(f"w1b{li}_{k}", wp[f"w1b{li}"][k * P:(k + 1) * P, :],
                           P, 2 * h + dout, BF16)
                    load_w(f"w2a{li}_{k}", wp[f"w2a{li}"][k * P:(k + 1) * P, :],
                           P, h, BF16)
                    load_w(f"w2b{li}_{k}", wp[f"w2b{li}"][k * P:(k + 1) * P, :],
                           P, dout, BF16)
                load_w(f"b1a{li}", wp[f"b1a{li}"][:, :], P, h // P, F32)
                load_w(f"b1bp{li}", wp[f"b1bp{li}"][:, :], dout, 1, F32)
                load_w(f"b1bsr{li}", wp[f"b1bsr{li}"][:, :], 1, h, BF16)
                load_w(f"b1bor{li}", wp[f"b1bor{li}"][:, :], 1, h, BF16)
                load_w(f"b2a{li}", wp[f"b2a{li}"][:, :], P, h // P, F32)
                load_w(f"b2b{li}", wp[f"b2b{li}"][:, :], dout, 1, F32)

            # ---------------- embedding phase ----------------
            NEB = -(-OSP // 512)
            with (
                tc.tile_pool(name="esb", bufs=3) as esb,
                tc.tile_pool(name="eps", bufs=3, space="PSUM") as eps,
            ):
                for b in range(NEB):
                    c0 = b * 512
                    w = min(512, OSP - c0)
                    xin = esb.tile([68, 512], BF16, tag="xin")
                    nc.sync.dma_start(out=xin[:, :w], in_=xt[:, c0:c0 + w])
                    pse = eps.tile([64, 512], F32, space="PSUM", tag="pse")
                    nc.tensor.matmul(out=pse[:, :w], lhsT=W["w_emb"][:], rhs=xin[:, :w],
                                     start=True, stop=True)
                    ebt = esb.tile([64, 512], BF16, tag="ebt")
                    nc.scalar.activation(out=ebt[:, :w], in_=pse[:, :w], func=PRELU,
                                         bias=W["b_emb"][:, :1], alpha=ALPHA)
                    for q in range(-(-w // P)):
                        qw = min(P, w - q * P)
                        ptr = eps.tile([P, 64], BF16, space="PSUM", tag="ptr")
                        nc.tensor.transpose(out=ptr[:qw, :], in_=ebt[:, q * P:q * P + qw],
                                            identity=ident_bf[:64, :64])
                        ent = esb.tile([P, 64], BF16, tag="ent")
                        nc.vector.tensor_copy(out=ent[:qw, :], in_=ptr[:qw, :])
                        nc.sync.dma_start(out=agins[0][c0 + q * P:c0 + q * P + qw, :],
                                          in_=ent[:qw, :])
            nc.gpsimd.collective_compute(
                "AllGather", mybir.AluOpType.bypass, replica_groups=GRPS,
                ins=[agins[0][:]], outs=[tabs[0][:]])

            # ---------------- layers ----------------
            for li, (din, h, dout) in enumerate(DIMS):
                tab_in = tabs[li]
                pred_in = preds[li]
                stage, send, recv = stages[li], sends[li], recvs[li]
                NH = h // P
                s_cols = (0, h)
                p_cols = (h, h + dout)
                o_cols = (h + dout, 2 * h + dout)

                # ---- phase A: triple MLP ----
                with (
                    tc.tile_pool(name=f"asb{li}", bufs=3) as asb,
                    tc.tile_pool(name=f"apstr{li}", bufs=2, space="PSUM") as aps_tr,
                    tc.tile_pool(name=f"apshid{li}", bufs=2, space="PSUM") as aps_hid,
                    tc.tile_pool(name=f"apsout{li}", bufs=2, space="PSUM") as aps_out,
                ):
                    for j in range(NB):
                        # gather 512 subject rows + 512 object rows
                        # (one [P,1]-offset indirect DMA per 128 rows: multi-
                        # column offset APs are not supported by HW SWDGE)
                        ge = asb.tile([P, 8 * din], BF16, tag="ge")
                        for g in range(8):
                            nc.gpsimd.indirect_dma_start(
                                out=ge[:, g * din:(g + 1) * din], out_offset=None,
                                in_=tab_in[:],
                                in_offset=bass.IndirectOffsetOnAxis(
                                    ap=W["sgog"][:, 8 * j + g:8 * j + g + 1], axis=0))
                        sT = asb.tile([din, 512], BF16, tag="sT")
                        oT = asb.tile([din, 512], BF16, tag="oT")
                        for g in range(8):
                            dst = sT if g < 4 else oT
                            gg = g % 4
                            ptr = aps_tr.tile([din, P], BF16, space="PSUM", tag="ptr")
                            nc.tensor.transpose(out=ptr[:], in_=ge[:, g * din:(g + 1) * din],
                                                identity=ident_bf[:])
                            nc.vector.tensor_copy(out=dst[:, gg * P:(gg + 1) * P],
                                                  in_=ptr[:])
                        pT = asb.tile([din, 512], BF16, tag="pT")
                        nc.sync.dma_start(out=pT[:], in_=pred_in[:din, 512 * j:512 * (j + 1)])

                        # hid
                        hidT = []
                        for mh in range(NH):
                            ph = aps_hid.tile([P, 512], F32, space="PSUM", tag="ph")
                            for ki, src in enumerate((sT, pT, oT)):
                                nc.tensor.matmul(
                                    out=ph[:],
                                    lhsT=W[f"w1a{li}_c{ki}"][:, mh * P:(mh + 1) * P],
                                    rhs=src[:],
                                    start=(ki == 0), stop=(ki == 2))
                            ht = asb.tile([P, 512], BF16, tag=f"hidT{mh}",
                                          name=f"hidT{mh}")
                            nc.scalar.activation(out=ht[:], in_=ph[:], func=PRELU,
                                                 bias=W[f"b1a{li}"][:, mh:mh + 1],
                                                 alpha=ALPHA)
                            hidT.append(ht)

                        # new_s / new_o (entry-major), bias via K=1 matmul,
                        # 1/count scale folded into the activation
                        ovs = asb.tile([P, 4 * h], BF16, tag="ovs")
                        ovo = asb.tile([P, 4 * h], BF16, tag="ovo")
                        for (cols, brow, ivname, dst) in (
                                (s_cols, f"b1bsr{li}", "invs", ovs),
                                (o_cols, f"b1bor{li}", "invo", ovo)):
                            for e in range(4):
                                po = aps_out.tile([P, 512], F32, space="PSUM", tag="po")
                                for k in range(NH):
                                    nc.tensor.matmul(
                                        out=po[:, :h],
                                        lhsT=hidT[k][:, e * P:(e + 1) * P],
                                        rhs=W[f"w1b{li}_{k}"][:, cols[0]:cols[1]],
                                        start=(k == 0), stop=False)
                                nc.tensor.matmul(
                                    out=po[:, :h], lhsT=ones_row[:, :],
                                    rhs=W[brow][:, :],
                                    start=False, stop=True)
                                nc.scalar.activation(
                                    out=dst[:, e * h:(e + 1) * h], in_=po[:, :h],
                                    func=PRELU, alpha=ALPHA,
                                    scale=W[ivname][:, 4 * j + e:4 * j + e + 1])
                        # sequential store of new_s rows
                        nc.sync.dma_start(
                            out=stage[512 * j:512 * (j + 1), :].rearrange(
                                "(e p) h -> p e h", e=4),
                            in_=ovs[:].rearrange("p (e h) -> p e h", e=4))
                        # scatter new_o rows into buckets
                        for e in range(4):
                            nc.gpsimd.indirect_dma_start(
                                out=send[:],
                                out_offset=bass.IndirectOffsetOnAxis(
                                    ap=W["ob"][:, 4 * j + e:4 * j + e + 1], axis=0),
                                in_=ovo[:, e * h:(e + 1) * h], in_offset=None)

                        # new_p (feature-major), not needed after last layer
                        if li < 3:
                            pp = aps_out.tile([P, 512], F32, space="PSUM", tag="po")
                            for k in range(NH):
                                nc.tensor.matmul(
                                    out=pp[:dout, :],
                                    lhsT=W[f"w1b{li}_{k}"][:, p_cols[0]:p_cols[1]],
                                    rhs=hidT[k][:],
                                    start=(k == 0), stop=(k == NH - 1))
                            pv = asb.tile([dout, 512], BF16, tag="pv")
                            nc.scalar.activation(out=pv[:], in_=pp[:dout, :], func=PRELU,
                                                 bias=W[f"b1bp{li}"][:, :1], alpha=ALPHA)
                            nc.sync.dma_start(
                                out=preds[li + 1][:, 512 * j:512 * (j + 1)], in_=pv[:])

                # ---- phase B: AllToAll ----
                nc.gpsimd.collective_compute(
                    "AllToAll", mybir.AluOpType.bypass, replica_groups=GRPS,
                    ins=[send[:]], outs=[recv[:]])

                # ---- phase C: pooling + object MLP ----
                with (
                    tc.tile_pool(name=f"csb{li}", bufs=3) as csb,
                    tc.tile_pool(name=f"cpool{li}", bufs=2, space="PSUM") as cps_pool,
                    tc.tile_pool(name=f"ctr{li}", bufs=2, space="PSUM") as cps_tr,
                    tc.tile_pool(name=f"cmlp{li}", bufs=2, space="PSUM") as cps_mlp,
                ):
                    ng = -(-NT // 4)
                    for grp in range(ng):
                        t0 = grp * 4
                        tn = min(4, NT - t0)
                        gw = tn * P
                        pooledT = [csb.tile([P, 512], BF16, tag=f"pooledT{k}",
                                            name=f"pooledT{k}")
                                   for k in range(NH)]
                        for tt in range(t0, t0 + tn):
                            # gathers: stage rows / recv rows for this tile
                            svals = csb.tile([P, PS * h], BF16, tag="svals")
                            for k in range(PS):
                                nc.gpsimd.indirect_dma_start(
                                    out=svals[:, k * h:(k + 1) * h], out_offset=None,
                                    in_=stage[:],
                                    in_offset=bass.IndirectOffsetOnAxis(
                                        ap=W["sids"][:, PS * tt + k:PS * tt + k + 1], axis=0))
                            ovals = csb.tile([P, PO * h], BF16, tag="ovals")
                            for k in range(PO):
                                nc.gpsimd.indirect_dma_start(
                                    out=ovals[:, k * h:(k + 1) * h], out_offset=None,
                                    in_=recv[:],
                                    in_offset=bass.IndirectOffsetOnAxis(
                                        ap=W["oids"][:, PO * tt + k:PO * tt + k + 1], axis=0))
                            pps = cps_pool.tile([P, 512], F32, space="PSUM", tag="pps")
                            nmm = 0
                            for (vals, PN, locs_t) in ((svals, PS, "slocs"),
                                                       (ovals, PO, "olocs")):
                                for k in range(PN):
                                    oh = csb.tile([P, P], BF16, tag="oh")
                                    nc.vector.tensor_tensor(
                                        out=oh[:],
                                        in0=W[locs_t][:, PN * tt + k:PN * tt + k + 1]
                                            .to_broadcast([P, P]),
                                        in1=iota[:], op=mybir.AluOpType.is_equal)
                                    for mh in range(NH):
                                        # start=True zeroes the whole 2KB
                                        # bank, so only the very first matmul
                                        # into this psum tile may set it
                                        nc.tensor.matmul(
                                            out=pps[:, mh * P:(mh + 1) * P],
                                            lhsT=vals[:, k * h + mh * P:k * h + (mh + 1) * P],
                                            rhs=oh[:],
                                            start=(nmm == 0 and mh == 0),
                                            stop=(nmm == (PS + PO) - 1),
                                            skip_group_check=True)
                                    nmm += 1
                            # pooled tile is already feature-major: psum layout
                            # [feat-chunk, subj] per mh slice
                            for mh in range(NH):
                                nc.vector.tensor_copy(
                                    out=pooledT[mh][:, (tt - t0) * P:(tt - t0 + 1) * P],
                                    in_=pps[:, mh * P:(mh + 1) * P])
                        # object MLP on gw objects
                        hid2 = []
                        for mh in range(NH):
                            p2 = cps_mlp.tile([P, 512], F32, space="PSUM", tag="p2")
                            for k in range(NH):
                                nc.tensor.matmul(
                                    out=p2[:, :gw],
                                    lhsT=W[f"w2a{li}_{k}"][:, mh * P:(mh + 1) * P],
                                    rhs=pooledT[k][:, :gw],
                                    start=(k == 0), stop=(k == NH - 1))
                            h2 = csb.tile([P, 512], BF16, tag=f"h2_{mh}",
                                          name=f"h2_{mh}")
                            nc.scalar.activation(out=h2[:, :gw], in_=p2[:, :gw],
                                                 func=PRELU,
                                                 bias=W[f"b2a{li}"][:, mh:mh + 1],
                                                 alpha=ALPHA)
                            hid2.append(h2)
                        pno = cps_mlp.tile([P, 512], F32, space="PSUM", tag="p2")
                        for k in range(NH):
                            nc.tensor.matmul(out=pno[:dout, :gw],
                                             lhsT=W[f"w2b{li}_{k}"][:],
                                             rhs=hid2[k][:, :gw],
                                             start=(k == 0), stop=(k == NH - 1))
                        noT = csb.tile([dout, 512], BF16, tag="noT")
                        nc.scalar.activation(out=noT[:, :gw], in_=pno[:dout, :gw],
                                             func=PRELU, bias=W[f"b2b{li}"][:, :1],
                                             alpha=ALPHA)
                        if li < 3:
                            for q in range(tn):
                                ptr3 = cps_tr.tile([P, P], BF16, space="PSUM", tag="ptr2")
                                nc.tensor.transpose(out=ptr3[:, :dout],
                                                    in_=noT[:, q * P:(q + 1) * P],
                                                    identity=ident_bf[:])
                                ent2 = csb.tile([P, P], BF16, tag="ent2")
                                nc.vector.tensor_copy(out=ent2[:, :dout],
                                                      in_=ptr3[:, :dout])
                                r0 = (t0 + q) * P
                                nc.sync.dma_start(out=agins[li + 1][r0:r0 + P, :],
                                                  in_=ent2[:, :dout])
                        else:
                            phd = cps_mlp.tile([4, 512], F32, space="PSUM", tag="phd")
                            nc.tensor.matmul(out=phd[:, :gw], lhsT=W["wbb"][:],
                                             rhs=noT[:, :gw], start=True, stop=True)
                            ho = csb.tile([4, 512], F32, tag="ho")
                            nc.scalar.activation(out=ho[:, :gw], in_=phd[:, :gw],
                                                 func=PRELU, bias=W["bbb"][:, :1],
                                                 alpha=ALPHA)
                            nc.sync.dma_start(out=out[:, t0 * P:t0 * P + gw],
                                              in_=ho[:, :gw])

                # ---- phase D: AllGather new object table ----
                if li < 3:
                    nc.gpsimd.collective_compute(
                        "AllGather", mybir.AluOpType.bypass, replica_groups=GRPS,
                        ins=[agins[li + 1][:]], outs=[tabs[li + 1][:]])

    nc.compile()
    return nc


# ---------------------------------------------------------------------------
# Entry point
# ---------------------------------------------------------------------------

_CACHE = {}


def kernel(**inputs) -> np.ndarray:
    cfg, in_maps = preprocess(inputs)
    key = tuple(sorted(cfg.items()))
    if key not in _CACHE:
        _CACHE[key] = build_kernel(cfg)
    nc = _CACHE[key]
    res = run_bass_kernel_spmd(nc, in_maps, list(range(NC)))
    O, OS = cfg["O"], cfg["OS"]
    full = np.zeros((4, O), np.float32)
    for c in range(NC):
        full[:, c * OS:(c + 1) * OS] = res.results[c]["out"][:, :OS]
    return np.ascontiguousarray(full.T)


# revision 3
# speedup vs baseline: 1.0810x; 1.0810x over previous
"""Trainium2 Bass kernel for nn_NDNRefinement (4-layer GNN message passing), v2.

Strategy (8 NeuronCores):
- Sort triples by s_idx on host; core c owns triples whose subject falls in
  [c*OS, (c+1)*OS). Subject-side pooled rows are written to a local `stage`
  buffer; object-side rows go to fixed-size per-destination buckets that are
  exchanged with one AllToAll per layer. Per-object MLPs are data-parallel
  over the object shard; the new object table is AllGathered between layers.
- v2 rework vs v1: one batched 512-row indirect gather (s+o rows together)
  and one batched 512-row indirect scatter per triple block; all index /
  schedule arrays preloaded into SBUF once; 1/count scaling folded into the
  producer-side activation scale; W1b bias added with a K=1 matmul so PSUM
  goes straight to the activation; pooling matmuls emit feature-major
  pooled tiles directly (no post-pool transposes).
"""

import numpy as np
import ml_dtypes

import concourse.bass as bass
import concourse.bacc as bacc
import concourse.tile as tile
from concourse import mybir
from concourse.bass_utils import run_bass_kernel_spmd
from concourse.masks import make_identity

BF16 = mybir.dt.bfloat16
F32 = mybir.dt.float32
I32 = mybir.dt.int32
P = 128
NC = 8
ALPHA = 0.2

# (din, h, dout) per layer
DIMS = [(64, 512, 128), (128, 512, 128), (128, 512, 128), (128, 128, 128)]


def _rup(x, m):
    return ((int(x) + m - 1) // m) * m


def _mega_layout(cfg):
    """Fixed ordering of every SBUF-resident tile, used both to pack the
    three mega input tensors on the host and to slice them in the kernel.
    Collapsing ~55 kernel parameters into 3 cuts per-call dispatch overhead.
    Returns [(name, rows, cols, dtype_key)] with dtype_key in {b, f, i}."""
    NB, NT, PS, PO = cfg["NB"], cfg["NT"], cfg["PS"], cfg["PO"]
    ents = [
        ("sgog", P, NB * 8, "i"),
        ("ob", P, NB * 4, "i"),
        ("sids", P, NT * PS, "i"),
        ("slocs", P, NT * PS, "i"),
        ("oids", P, NT * PO, "i"),
        ("olocs", P, NT * PO, "i"),
        ("invs", P, NB * 4, "f"),
        ("invo", P, NB * 4, "f"),
        ("w_emb", 68, 64, "b"),
        ("b_emb", 64, 1, "f"),
        ("wbb", P, 4, "b"),
        ("bbb", 4, 1, "f"),
    ]
    for li, (din, h, dout) in enumerate(DIMS):
        for ki in range(3):
            ents.append((f"w1a{li}_c{ki}", din, h, "b"))
        for k in range(h // P):
            ents.append((f"w1b{li}_{k}", P, 2 * h + dout, "b"))
            ents.append((f"w2a{li}_{k}", P, h, "b"))
            ents.append((f"w2b{li}_{k}", P, dout, "b"))
        ents.append((f"b1a{li}", P, h // P, "f"))
        ents.append((f"b1bp{li}", dout, 1, "f"))
        ents.append((f"b1bsr{li}", 1, h, "b"))
        ents.append((f"b1bor{li}", 1, h, "b"))
        ents.append((f"b2a{li}", P, h // P, "f"))
        ents.append((f"b2b{li}", dout, 1, "f"))
    return ents


# ---------------------------------------------------------------------------
# Host preprocessing
# ---------------------------------------------------------------------------

def preprocess(inputs):
    obj_vecs = np.asarray(inputs["obj_vecs"], np.float32)
    pred_vecs = np.asarray(inputs["pred_vecs"], np.float32)
    pred_boxes = np.asarray(inputs["pred_boxes"], np.float32)
    s_idx = np.asarray(inputs["s_idx"], np.int32)
    o_idx = np.asarray(inputs["o_idx"], np.int32)

    O = obj_vecs.shape[0]
    assert O % NC == 0
    OS = O // NC
    OSP = _rup(OS, P)
    NT = OSP // P
    OG = NC * OSP

    def gmap(idx):
        return ((idx // OS) * OSP + (idx % OS)).astype(np.int64)

    order = np.argsort(s_idx, kind="stable")
    s_sorted = s_idx[order]
    o_sorted = o_idx[order]
    bnd = np.searchsorted(s_sorted, np.arange(NC + 1) * OS)
    counts_c = np.diff(bnd)
    T_PC = max(_rup(counts_c.max(), 512), 512)
    NB = T_PC // 512

    cnt = np.bincount(s_idx, minlength=O) + np.bincount(o_idx, minlength=O)
    inv_cnt = (1.0 / np.maximum(cnt, 1)).astype(np.float32)

    percore = []
    maxB = 0
    max_s_load = 0
    for c in range(NC):
        sl = slice(bnd[c], bnd[c + 1])
        idxs = order[sl]
        n = len(idxs)
        s_c = s_sorted[sl]
        o_c = o_sorted[sl]
        d_c = (o_c // OS).astype(np.int64)
        ordb = np.lexsort((o_c, d_c))
        rank = np.empty(n, np.int64)
        d_srt = d_c[ordb]
        first = np.searchsorted(d_srt, np.arange(NC))
        rank[ordb] = np.arange(n) - first[d_srt]
        bc = np.bincount(d_c, minlength=NC)
        maxB = max(maxB, int(bc.max()) if n else 0)
        sload = np.bincount((s_c - c * OS) // P, minlength=NT)
        max_s_load = max(max_s_load, int(sload.max()) if n else 0)
        percore.append(dict(idxs=idxs, n=n, s_c=s_c, o_c=o_c, d_c=d_c,
                            rank=rank, bc=bc))

    n_pad_max = max(int(T_PC - pc["n"]) for pc in percore)
    S_B = _rup(maxB + n_pad_max // NC + 2, P)

    o_tiles_per_core = []
    max_o_load = 0
    for c in range(NC):
        rows_all, locs_all = [], []
        for d in range(NC):
            pc = percore[d]
            m = pc["d_c"] == c
            rows_all.append(d * S_B + pc["rank"][m])
            locs_all.append(pc["o_c"][m] - c * OS)
        rows_all = np.concatenate(rows_all)
        locs_all = np.concatenate(locs_all)
        oload = np.bincount(locs_all // P, minlength=NT)
        max_o_load = max(max_o_load, int(oload.max()) if len(locs_all) else 0)
        o_tiles_per_core.append((rows_all, locs_all))

    PS = max(1, -(-int(max_s_load) // P))
    PO = max(1, -(-int(max_o_load) // P))

    cfg = dict(O=O, OS=OS, OSP=OSP, NT=NT, OG=OG, T_PC=T_PC, NB=NB,
               S_B=S_B, PS=PS, PO=PO)

    bf = ml_dtypes.bfloat16
    shared = {}
    shared["w_emb"] = np.asarray(inputs["W_emb"], np.float32).astype(bf)
    shared["b_emb"] = np.asarray(inputs["b_emb"], np.float32).reshape(-1, 1)
    for li, (din, h, dout) in enumerate(DIMS):
        b1b = np.asarray(inputs[f"b1b{li}"], np.float32)
        w1a = np.asarray(inputs[f"W1a{li}"], np.float32).astype(bf)
        w1b = np.asarray(inputs[f"W1b{li}"], np.float32).astype(bf)
        w2a = np.asarray(inputs[f"W2a{li}"], np.float32).astype(bf)
        w2b = np.asarray(inputs[f"W2b{li}"], np.float32).astype(bf)
        for ki in range(3):
            shared[f"w1a{li}_c{ki}"] = w1a[ki * din:(ki + 1) * din, :]
        for k in range(h // P):
            shared[f"w1b{li}_{k}"] = w1b[k * P:(k + 1) * P, :]
            shared[f"w2a{li}_{k}"] = w2a[k * P:(k + 1) * P, :]
            shared[f"w2b{li}_{k}"] = w2b[k * P:(k + 1) * P, :]
        shared[f"b1a{li}"] = np.asarray(inputs[f"b1a{li}"], np.float32).reshape(-1, P).T.copy()
        shared[f"b1bp{li}"] = b1b[h:h + dout].reshape(-1, 1).copy()
        shared[f"b1bsr{li}"] = b1b[:h].astype(bf).reshape(1, h).copy()
        shared[f"b1bor{li}"] = b1b[h + dout:].astype(bf).reshape(1, h).copy()
        shared[f"b2a{li}"] = np.asarray(inputs[f"b2a{li}"], np.float32).reshape(-1, P).T.copy()
        shared[f"b2b{li}"] = np.asarray(inputs[f"b2b{li}"], np.float32).reshape(-1, 1).copy()
    shared["wbb"] = np.asarray(inputs["W_bb"], np.float32).astype(bf)
    shared["bbb"] = np.asarray(inputs["b_bb"], np.float32).reshape(-1, 1)
    layout = _mega_layout(cfg)

    x_full = np.concatenate([obj_vecs, pred_boxes], axis=1)

    in_maps = []
    for c in range(NC):
        pc = percore[c]
        idxs, n = pc["idxs"], pc["n"]
        m = {}
        tl = {}
        xT = np.zeros((68, OSP), bf)
        xT[:, :OS] = x_full[c * OS:(c + 1) * OS].T.astype(bf)
        m["xt"] = xT
        pT = np.zeros((64, T_PC), bf)
        pT[:, :n] = pred_vecs[idxs].T.astype(bf)
        m["pred0"] = pT

        # interleaved gather rows: block j, cols j*8+g: g<4 subject rows,
        # g>=4 object rows (global padded table ids); pads -> row 0
        sg_ = np.zeros((T_PC,), np.int64)
        sg_[:n] = gmap(pc["s_c"])
        og_ = np.zeros((T_PC,), np.int64)
        og_[:n] = gmap(pc["o_c"])
        sgog = np.zeros((NB, 8, P), np.int32)
        sgog[:, 0:4, :] = sg_.reshape(NB, 4, P)
        sgog[:, 4:8, :] = og_.reshape(NB, 4, P)
        tl["sgog"] = sgog.reshape(NB * 8, P).T.copy()

        # bucket scatter positions; pad entries spread over per-bucket headroom
        ob_ = np.empty((T_PC,), np.int64)
        ob_[:n] = (pc["d_c"] * S_B + pc["rank"])
        npad = T_PC - n
        if npad:
            i = np.arange(npad)
            d = i % NC
            slot = S_B - 1 - (i // NC)
            assert (slot >= pc["bc"][d]).all(), "trash slots collide with data"
            ob_[n:] = (d * S_B + slot)
        tl["ob"] = ob_.astype(np.int32).reshape(NB * 4, P).T.copy()

        # producer-side 1/count scales (pads -> 0)
        iv_s = np.zeros((T_PC,), np.float32)
        iv_s[:n] = inv_cnt[pc["s_c"]]
        iv_o = np.zeros((T_PC,), np.float32)
        iv_o[:n] = inv_cnt[pc["o_c"]]
        tl["invs"] = iv_s.reshape(NB * 4, P).T.copy()
        tl["invo"] = iv_o.reshape(NB * 4, P).T.copy()

        # s-pool schedule: ids into stage rows, locs relative to tile
        s_loc = pc["s_c"] - c * OS
        sids = np.zeros((NT, PS, P), np.int32)
        slocs = np.full((NT, PS, P), -1, np.int32)
        tstart = np.searchsorted(s_loc, np.arange(NT + 1) * P)
        for t in range(NT):
            a, b = int(tstart[t]), int(tstart[t + 1])
            k = b - a
            fi = np.zeros((PS * P,), np.int32)
            fl = np.full((PS * P,), -1, np.int32)
            fi[:k] = np.arange(a, b, dtype=np.int32)
            fl[:k] = (s_loc[a:b] - t * P).astype(np.int32)
            sids[t] = fi.reshape(PS, P)
            slocs[t] = fl.reshape(PS, P)
        tl["sids"] = sids.reshape(NT * PS, P).T.copy()
        tl["slocs"] = slocs.reshape(NT * PS, P).T.copy()

        # o-pool schedule: ids into recv rows
        rows_all, locs_all = o_tiles_per_core[c]
        oids = np.zeros((NT, PO, P), np.int32)
        olocs = np.full((NT, PO, P), -1, np.int32)
        tsel = locs_all // P
        for t in range(NT):
            msk = tsel == t
            k = int(msk.sum())
            fi = np.zeros((PO * P,), np.int32)
            fl = np.full((PO * P,), -1, np.int32)
            fi[:k] = rows_all[msk]
            fl[:k] = (locs_all[msk] - t * P)
            oids[t] = fi.reshape(PO, P)
            olocs[t] = fl.reshape(PO, P)
        tl["oids"] = oids.reshape(NT * PO, P).T.copy()
        tl["olocs"] = olocs.reshape(NT * PO, P).T.copy()

        tl.update(shared)
        # pack every resident tile into one tensor per dtype
        npdt = {"b": ml_dtypes.bfloat16, "f": np.float32, "i": np.int32}
        tot = {d: 0 for d in npdt}
        for (_, _, cols, d) in layout:
            tot[d] += cols
        megas = {d: np.zeros((P, tot[d]), npdt[d]) for d in npdt}
        off = {d: 0 for d in npdt}
        for (name, rows, cols, d) in layout:
            arr = np.asarray(tl[name], npdt[d])
            assert arr.shape == (rows, cols), (name, arr.shape, rows, cols)
            megas[d][:rows, off[d]:off[d] + cols] = arr
            off[d] += cols
        m["megab"], m["megaf"], m["megai"] = megas["b"], megas["f"], megas["i"]
        in_maps.append(m)

    return cfg, in_maps


# ---------------------------------------------------------------------------
# Kernel builder
# ---------------------------------------------------------------------------

def build_kernel(cfg):
    OSP, NT, OG = cfg["OSP"], cfg["NT"], cfg["OG"]
    T_PC, NB, S_B = cfg["T_PC"], cfg["NB"], cfg["S_B"]
    PS, PO = cfg["PS"], cfg["PO"]

    nc = bacc.Bacc("TRN2", target_bir_lowering=False, debug=False,
                   num_devices=NC)

    # ---- parameters (all resident tiles packed into 3 mega tensors) ----
    layout = _mega_layout(cfg)
    tot = {"b": 0, "f": 0, "i": 0}
    for (_, _, cols, d) in layout:
        tot[d] += cols
    xt = nc.declare_dram_parameter("xt", [68, OSP], BF16, isOutput=False)
    pred0 = nc.declare_dram_parameter("pred0", [64, T_PC], BF16, isOutput=False)
    megas = {
        "b": nc.declare_dram_parameter("megab", [P, tot["b"]], BF16, isOutput=False),
        "f": nc.declare_dram_parameter("megaf", [P, tot["f"]], F32, isOutput=False),
        "i": nc.declare_dram_parameter("megai", [P, tot["i"]], I32, isOutput=False),
    }
    MDT = {"b": BF16, "f": F32, "i": I32}

    out = nc.declare_dram_parameter("out", [4, OSP], F32, isOutput=True)

    # ---- internal DRAM ----
    tabs = [nc.dram_tensor("tab0", [OG, 64], BF16, addr_space="Shared")]
    for li in range(1, 4):
        tabs.append(nc.dram_tensor(f"tab{li}", [OG, P], BF16, addr_space="Shared"))
    preds = [pred0]
    for li in range(1, 4):
        preds.append(nc.dram_tensor(f"pred{li}", [P, T_PC], BF16))
    stages, sends, recvs, agins = [], [], [], []
    for li, (din, h, dout) in enumerate(DIMS):
        stages.append(nc.dram_tensor(f"stage{li}", [T_PC, h], BF16))
        sends.append(nc.dram_tensor(f"send{li}", [NC * S_B, h], BF16))
        recvs.append(nc.dram_tensor(f"recv{li}", [NC * S_B, h], BF16))
    agins.append(nc.dram_tensor("agin_e", [OSP, 64], BF16))
    for li in range(3):
        agins.append(nc.dram_tensor(f"agin{li}", [OSP, P], BF16))

    PRELU = mybir.ActivationFunctionType.Prelu
    GRPS = [list(range(NC))]

    with tile.TileContext(nc) as tc:
        with tc.tile_pool(name="cst", bufs=1) as cst:
            # constants
            ident = cst.tile([P, P], F32)
            make_identity(nc, ident[:])
            ident_bf = cst.tile([P, P], BF16)
            nc.vector.tensor_copy(out=ident_bf[:], in_=ident[:])
            iota = cst.tile([P, P], I32)
            nc.gpsimd.iota(iota[:], pattern=[[1, P]], base=0, channel_multiplier=0)
            ones_row = cst.tile([1, P], BF16)
            nc.vector.memset(ones_row[:], 1.0)

            W = {}
            moff = {"b": 0, "f": 0, "i": 0}
            for (name, rows, cols, d) in layout:
                t = cst.tile([rows, cols], MDT[d], tag=name)
                nc.sync.dma_start(
                    out=t[:], in_=megas[d][0:rows, moff[d]:moff[d] + cols])
                W[name] = t
                moff[d] += cols

            # ---------------- embedding phase ----------------
            NEB = -(-OSP // 512)
            with (
                tc.tile_pool(name="esb", bufs=3) as esb,
                tc.tile_pool(name="eps", bufs=3, space="PSUM") as eps,
            ):
                for b in range(NEB):
                    c0 = b * 512
                    w = min(512, OSP - c0)
                    xin = esb.tile([68, 512], BF16, tag="xin")
                    nc.sync.dma_start(out=xin[:, :w], in_=xt[:, c0:c0 + w])
                    pse = eps.tile([64, 512], F32, space="PSUM", tag="pse")
                    nc.tensor.matmul(out=pse[:, :w], lhsT=W["w_emb"][:], rhs=xin[:, :w],
                                     start=True, stop=True)
                    ebt = esb.tile([64, 512], BF16, tag="ebt")
                    nc.scalar.activation(out=ebt[:, :w], in_=pse[:, :w], func=PRELU,
                                         bias=W["b_emb"][:, :1], alpha=ALPHA)
                    for q in range(-(-w // P)):
                        qw = min(P, w - q * P)
                        ptr = eps.tile([P, 64], BF16, space="PSUM", tag="ptr")
                        nc.tensor.transpose(out=ptr[:qw, :], in_=ebt[:, q * P:q * P + qw],
                                            identity=ident_bf[:64, :64])
                        ent = esb.tile([P, 64], BF16, tag="ent")
                        nc.vector.tensor_copy(out=ent[:qw, :], in_=ptr[:qw, :])
                        nc.sync.dma_start(out=agins[0][c0 + q * P:c0 + q * P + qw, :],
                                          in_=ent[:qw, :])
            nc.gpsimd.collective_compute(
                "AllGather", mybir.AluOpType.bypass, replica_groups=GRPS,
                ins=[agins[0][:]], outs=[tabs[0][:]])

            # ---------------- layers ----------------
            for li, (din, h, dout) in enumerate(DIMS):
                tab_in = tabs[li]
                pred_in = preds[li]
                stage, send, recv = stages[li], sends[li], recvs[li]
                NH = h // P
                s_cols = (0, h)
                p_cols = (h, h + dout)
                o_cols = (h + dout, 2 * h + dout)

                # ---- phase A: triple MLP ----
                with (
                    tc.tile_pool(name=f"asb{li}", bufs=3) as asb,
                    tc.tile_pool(name=f"apstr{li}", bufs=2, space="PSUM") as aps_tr,
                    tc.tile_pool(name=f"apshid{li}", bufs=3, space="PSUM") as aps_hid,
                    tc.tile_pool(name=f"apsout{li}", bufs=3, space="PSUM") as aps_out,
                ):
                    for j in range(NB):
                        # gather 512 subject rows + 512 object rows
                        # (one [P,1]-offset indirect DMA per 128 rows: multi-
                        # column offset APs are not supported by HW SWDGE)
                        ge = asb.tile([P, 8 * din], BF16, tag="ge")
                        for g in range(8):
                            nc.gpsimd.indirect_dma_start(
                                out=ge[:, g * din:(g + 1) * din], out_offset=None,
                                in_=tab_in[:],
                                in_offset=bass.IndirectOffsetOnAxis(
                                    ap=W["sgog"][:, 8 * j + g:8 * j + g + 1], axis=0))
                        sT = asb.tile([din, 512], BF16, tag="sT")
                        oT = asb.tile([din, 512], BF16, tag="oT")
                        for g in range(8):
                            dst = sT if g < 4 else oT
                            gg = g % 4
                            ptr = aps_tr.tile([din, P], BF16, space="PSUM", tag="ptr")
                            nc.tensor.transpose(out=ptr[:], in_=ge[:, g * din:(g + 1) * din],
                                                identity=ident_bf[:])
                            nc.vector.tensor_copy(out=dst[:, gg * P:(gg + 1) * P],
                                                  in_=ptr[:])
                        pT = asb.tile([din, 512], BF16, tag="pT")
                        nc.sync.dma_start(out=pT[:], in_=pred_in[:din, 512 * j:512 * (j + 1)])

                        # hid
                        hidT = []
                        for mh in range(NH):
                            ph = aps_hid.tile([P, 512], F32, space="PSUM", tag="ph")
                            for ki, src in enumerate((sT, pT, oT)):
                                nc.tensor.matmul(
                                    out=ph[:],
                                    lhsT=W[f"w1a{li}_c{ki}"][:, mh * P:(mh + 1) * P],
                                    rhs=src[:],
                                    start=(ki == 0), stop=(ki == 2))
                            ht = asb.tile([P, 512], BF16, tag=f"hidT{mh}",
                                          name=f"hidT{mh}")
                            nc.scalar.activation(out=ht[:], in_=ph[:], func=PRELU,
                                                 bias=W[f"b1a{li}"][:, mh:mh + 1],
                                                 alpha=ALPHA)
                            hidT.append(ht)

                        # new_s / new_o (entry-major), bias via K=1 matmul,
                        # 1/count scale folded into the activation
                        ovs = asb.tile([P, 4 * h], BF16, tag="ovs")
                        ovo = asb.tile([P, 4 * h], BF16, tag="ovo")
                        for (cols, brow, ivname, dst) in (
                                (s_cols, f"b1bsr{li}", "invs", ovs),
                                (o_cols, f"b1bor{li}", "invo", ovo)):
                            for e in range(4):
                                po = aps_out.tile([P, 512], F32, space="PSUM", tag="po")
                                for k in range(NH):
                                    nc.tensor.matmul(
                                        out=po[:, :h],
                                        lhsT=hidT[k][:, e * P:(e + 1) * P],
                                        rhs=W[f"w1b{li}_{k}"][:, cols[0]:cols[1]],
                                        start=(k == 0), stop=False)
                                nc.tensor.matmul(
                                    out=po[:, :h], lhsT=ones_row[:, :],
                                    rhs=W[brow][:, :],
                                    start=False, stop=True)
                                nc.scalar.activation(
                                    out=dst[:, e * h:(e + 1) * h], in_=po[:, :h],
                                    func=PRELU, alpha=ALPHA,
                                    scale=W[ivname][:, 4 * j + e:4 * j + e + 1])
                        # sequential store of new_s rows
                        nc.sync.dma_start(
                            out=stage[512 * j:512 * (j + 1), :].rearrange(
                                "(e p) h -> p e h", e=4),
                            in_=ovs[:].rearrange("p (e h) -> p e h", e=4))
                        # scatter new_o rows into buckets
                        for e in range(4):
                            nc.gpsimd.indirect_dma_start(
                                out=send[:],
                                out_offset=bass.IndirectOffsetOnAxis(
                                    ap=W["ob"][:, 4 * j + e:4 * j + e + 1], axis=0),
                                in_=ovo[:, e * h:(e + 1) * h], in_offset=None)

                        # new_p (feature-major), not needed after last layer
                        if li < 3:
                            pp = aps_out.tile([P, 512], F32, space="PSUM", tag="po")
                            for k in range(NH):
                                nc.tensor.matmul(
                                    out=pp[:dout, :],
                                    lhsT=W[f"w1b{li}_{k}"][:, p_cols[0]:p_cols[1]],
                                    rhs=hidT[k][:],
                                    start=(k == 0), stop=(k == NH - 1))
                            pv = asb.tile([dout, 512], BF16, tag="pv")
                            nc.scalar.activation(out=pv[:], in_=pp[:dout, :], func=PRELU,
                                                 bias=W[f"b1bp{li}"][:, :1], alpha=ALPHA)
                            nc.sync.dma_start(
                                out=preds[li + 1][:, 512 * j:512 * (j + 1)], in_=pv[:])

                # ---- phase B: AllToAll ----
                nc.gpsimd.collective_compute(
                    "AllToAll", mybir.AluOpType.bypass, replica_groups=GRPS,
                    ins=[send[:]], outs=[recv[:]])

                # ---- phase C: pooling + object MLP ----
                with (
                    tc.tile_pool(name=f"csb{li}", bufs=3) as csb,
                    tc.tile_pool(name=f"cpool{li}", bufs=2, space="PSUM") as cps_pool,
                    tc.tile_pool(name=f"ctr{li}", bufs=2, space="PSUM") as cps_tr,
                    tc.tile_pool(name=f"cmlp{li}", bufs=2, space="PSUM") as cps_mlp,
                ):
                    ng = -(-NT // 4)
                    for grp in range(ng):
                        t0 = grp * 4
                        tn = min(4, NT - t0)
                        gw = tn * P
                        pooledT = [csb.tile([P, 512], BF16, tag=f"pooledT{k}",
                                            name=f"pooledT{k}")
                                   for k in range(NH)]
                        for tt in range(t0, t0 + tn):
                            # gathers: stage rows / recv rows for this tile
                            svals = csb.tile([P, PS * h], BF16, tag="svals")
                            for k in range(PS):
                                nc.gpsimd.indirect_dma_start(
                                    out=svals[:, k * h:(k + 1) * h], out_offset=None,
                                    in_=stage[:],
                                    in_offset=bass.IndirectOffsetOnAxis(
                                        ap=W["sids"][:, PS * tt + k:PS * tt + k + 1], axis=0))
                            ovals = csb.tile([P, PO * h], BF16, tag="ovals")
                            for k in range(PO):
                                nc.gpsimd.indirect_dma_start(
                                    out=ovals[:, k * h:(k + 1) * h], out_offset=None,
                                    in_=recv[:],
                                    in_offset=bass.IndirectOffsetOnAxis(
                                        ap=W["oids"][:, PO * tt + k:PO * tt + k + 1], axis=0))
                            pps = cps_pool.tile([P, 512], F32, space="PSUM", tag="pps")
                            nmm = 0
                            for (vals, PN, locs_t) in ((svals, PS, "slocs"),
                                                       (ovals, PO, "olocs")):
                                for k in range(PN):
                                    oh = csb.tile([P, P], BF16, tag="oh")
                                    nc.vector.tensor_tensor(
                                        out=oh[:],
                                        in0=W[locs_t][:, PN * tt + k:PN * tt + k + 1]
                                            .to_broadcast([P, P]),
                                        in1=iota[:], op=mybir.AluOpType.is_equal)
                                    for mh in range(NH):
                                        # start=True zeroes the whole 2KB
                                        # bank, so only the very first matmul
                                        # into this psum tile may set it
                                        nc.tensor.matmul(
                                            out=pps[:, mh * P:(mh + 1) * P],
                                            lhsT=vals[:, k * h + mh * P:k * h + (mh + 1) * P],
                                            rhs=oh[:],
                                            start=(nmm == 0 and mh == 0),
                                            stop=(nmm == (PS + PO) - 1),
                                            skip_group_check=True)
                                    nmm += 1
                            # pooled tile is already feature-major: psum layout
                            # [feat-chunk, subj] per mh slice
                            for mh in range(NH):
                                nc.vector.tensor_copy(
                                    out=pooledT[mh][:, (tt - t0) * P:(tt - t0 + 1) * P],
                                    in_=pps[:, mh * P:(mh + 1) * P])
                        # object MLP on gw objects
                        hid2 = []
                        for mh in range(NH):
                            p2 = cps_mlp.tile([P, 512], F32, space="PSUM", tag="p2")
                            for k in range(NH):
                                nc.tensor.matmul(
                                    out=p2[:, :gw],
                                    lhsT=W[f"w2a{li}_{k}"][:, mh * P:(mh + 1) * P],
                                    rhs=pooledT[k][:, :gw],
                                    start=(k == 0), stop=(k == NH - 1))
                            h2 = csb.tile([P, 512], BF16, tag=f"h2_{mh}",
                                          name=f"h2_{mh}")
                            nc.scalar.activation(out=h2[:, :gw], in_=p2[:, :gw],
                                                 func=PRELU,
                                                 bias=W[f"b2a{li}"][:, mh:mh + 1],
                                                 alpha=ALPHA)
                            hid2.append(h2)
                        pno = cps_mlp.tile([P, 512], F32, space="PSUM", tag="p2")
                        for k in range(NH):
                            nc.tensor.matmul(out=pno[:dout, :gw],
                                             lhsT=W[f"w2b{li}_{k}"][:],
                                             rhs=hid2[k][:, :gw],
                                             start=(k == 0), stop=(k == NH - 1))
                        noT = csb.tile([dout, 512], BF16, tag="noT")
                        nc.scalar.activation(out=noT[:, :gw], in_=pno[:dout, :gw],
                                             func=PRELU, bias=W[f"b2b{li}"][:, :1],
                                             alpha=ALPHA)
                        if li < 3:
                            for q in range(tn):
                                ptr3 = cps_tr.tile([P, P], BF16, space="PSUM", tag="ptr2")
                                nc.tensor.transpose(out=ptr3[:, :dout],
                                                    in_=noT[:, q * P:(q + 1) * P],
                                                    identity=ident_bf[:])
                                ent2 = csb.tile([P, P], BF16, tag="ent2")
                                nc.vector.tensor_copy(out=ent2[:, :dout],
                                                      in_=ptr3[:, :dout])
                                r0 = (t0 + q) * P
                                nc.sync.dma_start(out=agins[li + 1][r0:r0 + P, :],
                                                  in_=ent2[:, :dout])
                        else:
                            phd = cps_mlp.tile([4, 512], F32, space="PSUM", tag="phd")
                            nc.tensor.matmul(out=phd[:, :gw], lhsT=W["wbb"][:],
                                             rhs=noT[:, :gw], start=True, stop=True)
                            ho = csb.tile([4, 512], F32, tag="ho")
                            nc.scalar.activation(out=ho[:, :gw], in_=phd[:, :gw],
                                                 func=PRELU, bias=W["bbb"][:, :1],
                                                 alpha=ALPHA)
                            nc.sync.dma_start(out=out[:, t0 * P:t0 * P + gw],
                                              in_=ho[:, :gw])

                # ---- phase D: AllGather new object table ----
                if li < 3:
                    nc.gpsimd.collective_compute(
                        "AllGather", mybir.AluOpType.bypass, replica_groups=GRPS,
                        ins=[agins[li + 1][:]], outs=[tabs[li + 1][:]])

    nc.compile()
    return nc


# ---------------------------------------------------------------------------
# Entry point
# ---------------------------------------------------------------------------

_CACHE = {}


def kernel(**inputs) -> np.ndarray:
    cfg, in_maps = preprocess(inputs)
    key = tuple(sorted(cfg.items()))
    if key not in _CACHE:
        _CACHE[key] = build_kernel(cfg)
    nc = _CACHE[key]
    res = run_bass_kernel_spmd(nc, in_maps, list(range(NC)))
    O, OS = cfg["O"], cfg["OS"]
    full = np.zeros((4, O), np.float32)
    for c in range(NC):
        full[:, c * OS:(c + 1) * OS] = res.results[c]["out"][:, :OS]
    return np.ascontiguousarray(full.T)


# revision 4
# speedup vs baseline: 1.1639x; 1.0767x over previous
"""Trainium2 Bass kernel for nn_NDNRefinement (4-layer GNN message passing), v2.

Strategy (8 NeuronCores):
- Sort triples by s_idx on host; core c owns triples whose subject falls in
  [c*OS, (c+1)*OS). Subject-side pooled rows are written to a local `stage`
  buffer; object-side rows go to fixed-size per-destination buckets that are
  exchanged with one AllToAll per layer. Per-object MLPs are data-parallel
  over the object shard; the new object table is AllGathered between layers.
- v2 rework vs v1: one batched 512-row indirect gather (s+o rows together)
  and one batched 512-row indirect scatter per triple block; all index /
  schedule arrays preloaded into SBUF once; 1/count scaling folded into the
  producer-side activation scale; W1b bias added with a K=1 matmul so PSUM
  goes straight to the activation; pooling matmuls emit feature-major
  pooled tiles directly (no post-pool transposes).
"""

import numpy as np
import ml_dtypes

import concourse.bass as bass
import concourse.bacc as bacc
import concourse.tile as tile
from concourse import mybir
from concourse.bass_utils import run_bass_kernel_spmd
from concourse.masks import make_identity

BF16 = mybir.dt.bfloat16
F32 = mybir.dt.float32
I32 = mybir.dt.int32
P = 128
NC = 8
ALPHA = 0.2

# (din, h, dout) per layer
DIMS = [(64, 512, 128), (128, 512, 128), (128, 512, 128), (128, 128, 128)]


def _rup(x, m):
    return ((int(x) + m - 1) // m) * m


def _mega_layout(cfg):
    """Fixed ordering of every SBUF-resident tile, used both to pack the
    three mega input tensors on the host and to slice them in the kernel.
    Collapsing ~55 kernel parameters into 3 cuts per-call dispatch overhead.
    Returns [(name, rows, cols, dtype_key)] with dtype_key in {b, f, i}."""
    NB, NT, PS, PO = cfg["NB"], cfg["NT"], cfg["PS"], cfg["PO"]
    ents = [
        ("sgog", P, NB * 8, "i"),
        ("ob", P, NB * 4, "i"),
        ("sids", P, NT * PS, "i"),
        ("slocs", P, NT * PS, "i"),
        ("oids", P, NT * PO, "i"),
        ("olocs", P, NT * PO, "i"),
        ("invs", P, NB * 4, "f"),
        ("invo", P, NB * 4, "f"),
        ("w_emb", 68, 64, "b"),
        ("b_emb", 64, 1, "f"),
        ("wbb", P, 4, "b"),
        ("bbb", 4, 1, "f"),
    ]
    for li, (din, h, dout) in enumerate(DIMS):
        for ki in range(3):
            ents.append((f"w1a{li}_c{ki}", din, h, "b"))
        for k in range(h // P):
            ents.append((f"w1b{li}_{k}", P, 2 * h + dout, "b"))
            ents.append((f"w2a{li}_{k}", P, h, "b"))
            ents.append((f"w2b{li}_{k}", P, dout, "b"))
        ents.append((f"b1a{li}", P, h // P, "f"))
        ents.append((f"b1bp{li}", dout, 1, "f"))
        ents.append((f"b1bsr{li}", 1, h, "b"))
        ents.append((f"b1bor{li}", 1, h, "b"))
        ents.append((f"b2a{li}", P, h // P, "f"))
        ents.append((f"b2b{li}", dout, 1, "f"))
    return ents


# ---------------------------------------------------------------------------
# Host preprocessing
# ---------------------------------------------------------------------------

def preprocess(inputs):
    obj_vecs = np.asarray(inputs["obj_vecs"], np.float32)
    pred_vecs = np.asarray(inputs["pred_vecs"], np.float32)
    pred_boxes = np.asarray(inputs["pred_boxes"], np.float32)
    s_idx = np.asarray(inputs["s_idx"], np.int32)
    o_idx = np.asarray(inputs["o_idx"], np.int32)

    O = obj_vecs.shape[0]
    assert O % NC == 0
    OS = O // NC
    OSP = _rup(OS, P)
    NT = OSP // P
    OG = NC * OSP

    def gmap(idx):
        return ((idx // OS) * OSP + (idx % OS)).astype(np.int64)

    order = np.argsort(s_idx, kind="stable")
    s_sorted = s_idx[order]
    o_sorted = o_idx[order]
    bnd = np.searchsorted(s_sorted, np.arange(NC + 1) * OS)
    counts_c = np.diff(bnd)
    T_PC = max(_rup(counts_c.max(), 512), 512)
    NB = T_PC // 512

    cnt = np.bincount(s_idx, minlength=O) + np.bincount(o_idx, minlength=O)
    inv_cnt = (1.0 / np.maximum(cnt, 1)).astype(np.float32)

    percore = []
    maxB = 0
    max_s_load = 0
    for c in range(NC):
        sl = slice(bnd[c], bnd[c + 1])
        idxs = order[sl]
        n = len(idxs)
        s_c = s_sorted[sl]
        o_c = o_sorted[sl]
        d_c = (o_c // OS).astype(np.int64)
        ordb = np.lexsort((o_c, d_c))
        rank = np.empty(n, np.int64)
        d_srt = d_c[ordb]
        first = np.searchsorted(d_srt, np.arange(NC))
        rank[ordb] = np.arange(n) - first[d_srt]
        bc = np.bincount(d_c, minlength=NC)
        maxB = max(maxB, int(bc.max()) if n else 0)
        sload = np.bincount((s_c - c * OS) // P, minlength=NT)
        max_s_load = max(max_s_load, int(sload.max()) if n else 0)
        percore.append(dict(idxs=idxs, n=n, s_c=s_c, o_c=o_c, d_c=d_c,
                            rank=rank, bc=bc))

    n_pad_max = max(int(T_PC - pc["n"]) for pc in percore)
    S_B = _rup(maxB + n_pad_max // NC + 2, P)

    o_tiles_per_core = []
    max_o_load = 0
    for c in range(NC):
        rows_all, locs_all = [], []
        for d in range(NC):
            pc = percore[d]
            m = pc["d_c"] == c
            rows_all.append(d * S_B + pc["rank"][m])
            locs_all.append(pc["o_c"][m] - c * OS)
        rows_all = np.concatenate(rows_all)
        locs_all = np.concatenate(locs_all)
        oload = np.bincount(locs_all // P, minlength=NT)
        max_o_load = max(max_o_load, int(oload.max()) if len(locs_all) else 0)
        o_tiles_per_core.append((rows_all, locs_all))

    PS = max(1, -(-int(max_s_load) // P))
    PO = max(1, -(-int(max_o_load) // P))

    cfg = dict(O=O, OS=OS, OSP=OSP, NT=NT, OG=OG, T_PC=T_PC, NB=NB,
               S_B=S_B, PS=PS, PO=PO)

    bf = ml_dtypes.bfloat16
    shared = {}
    shared["w_emb"] = np.asarray(inputs["W_emb"], np.float32).astype(bf)
    shared["b_emb"] = np.asarray(inputs["b_emb"], np.float32).reshape(-1, 1)
    for li, (din, h, dout) in enumerate(DIMS):
        b1b = np.asarray(inputs[f"b1b{li}"], np.float32)
        w1a = np.asarray(inputs[f"W1a{li}"], np.float32).astype(bf)
        w1b = np.asarray(inputs[f"W1b{li}"], np.float32).astype(bf)
        w2a = np.asarray(inputs[f"W2a{li}"], np.float32).astype(bf)
        w2b = np.asarray(inputs[f"W2b{li}"], np.float32).astype(bf)
        for ki in range(3):
            shared[f"w1a{li}_c{ki}"] = w1a[ki * din:(ki + 1) * din, :]
        for k in range(h // P):
            shared[f"w1b{li}_{k}"] = w1b[k * P:(k + 1) * P, :]
            shared[f"w2a{li}_{k}"] = w2a[k * P:(k + 1) * P, :]
            shared[f"w2b{li}_{k}"] = w2b[k * P:(k + 1) * P, :]
        shared[f"b1a{li}"] = np.asarray(inputs[f"b1a{li}"], np.float32).reshape(-1, P).T.copy()
        shared[f"b1bp{li}"] = b1b[h:h + dout].reshape(-1, 1).copy()
        shared[f"b1bsr{li}"] = b1b[:h].astype(bf).reshape(1, h).copy()
        shared[f"b1bor{li}"] = b1b[h + dout:].astype(bf).reshape(1, h).copy()
        shared[f"b2a{li}"] = np.asarray(inputs[f"b2a{li}"], np.float32).reshape(-1, P).T.copy()
        shared[f"b2b{li}"] = np.asarray(inputs[f"b2b{li}"], np.float32).reshape(-1, 1).copy()
    shared["wbb"] = np.asarray(inputs["W_bb"], np.float32).astype(bf)
    shared["bbb"] = np.asarray(inputs["b_bb"], np.float32).reshape(-1, 1)
    layout = _mega_layout(cfg)

    x_full = np.concatenate([obj_vecs, pred_boxes], axis=1)

    in_maps = []
    for c in range(NC):
        pc = percore[c]
        idxs, n = pc["idxs"], pc["n"]
        m = {}
        tl = {}
        xT = np.zeros((68, OSP), bf)
        xT[:, :OS] = x_full[c * OS:(c + 1) * OS].T.astype(bf)
        m["xt"] = xT
        pT = np.zeros((64, T_PC), bf)
        pT[:, :n] = pred_vecs[idxs].T.astype(bf)
        m["pred0"] = pT

        # interleaved gather rows: block j, cols j*8+g: g<4 subject rows,
        # g>=4 object rows (global padded table ids); pads -> row 0
        sg_ = np.zeros((T_PC,), np.int64)
        sg_[:n] = gmap(pc["s_c"])
        og_ = np.zeros((T_PC,), np.int64)
        og_[:n] = gmap(pc["o_c"])
        sgog = np.zeros((NB, 8, P), np.int32)
        sgog[:, 0:4, :] = sg_.reshape(NB, 4, P)
        sgog[:, 4:8, :] = og_.reshape(NB, 4, P)
        tl["sgog"] = sgog.reshape(NB * 8, P).T.copy()

        # bucket scatter positions; pad entries spread over per-bucket headroom
        ob_ = np.empty((T_PC,), np.int64)
        ob_[:n] = (pc["d_c"] * S_B + pc["rank"])
        npad = T_PC - n
        if npad:
            i = np.arange(npad)
            d = i % NC
            slot = S_B - 1 - (i // NC)
            assert (slot >= pc["bc"][d]).all(), "trash slots collide with data"
            ob_[n:] = (d * S_B + slot)
        tl["ob"] = ob_.astype(np.int32).reshape(NB * 4, P).T.copy()

        # producer-side 1/count scales (pads -> 0)
        iv_s = np.zeros((T_PC,), np.float32)
        iv_s[:n] = inv_cnt[pc["s_c"]]
        iv_o = np.zeros((T_PC,), np.float32)
        iv_o[:n] = inv_cnt[pc["o_c"]]
        tl["invs"] = iv_s.reshape(NB * 4, P).T.copy()
        tl["invo"] = iv_o.reshape(NB * 4, P).T.copy()

        # s-pool schedule: ids into stage rows, locs relative to tile
        s_loc = pc["s_c"] - c * OS
        sids = np.zeros((NT, PS, P), np.int32)
        slocs = np.full((NT, PS, P), -1, np.int32)
        tstart = np.searchsorted(s_loc, np.arange(NT + 1) * P)
        for t in range(NT):
            a, b = int(tstart[t]), int(tstart[t + 1])
            k = b - a
            fi = np.zeros((PS * P,), np.int32)
            fl = np.full((PS * P,), -1, np.int32)
            fi[:k] = np.arange(a, b, dtype=np.int32)
            fl[:k] = (s_loc[a:b] - t * P).astype(np.int32)
            sids[t] = fi.reshape(PS, P)
            slocs[t] = fl.reshape(PS, P)
        tl["sids"] = sids.reshape(NT * PS, P).T.copy()
        tl["slocs"] = slocs.reshape(NT * PS, P).T.copy()

        # o-pool schedule: ids into recv rows
        rows_all, locs_all = o_tiles_per_core[c]
        oids = np.zeros((NT, PO, P), np.int32)
        olocs = np.full((NT, PO, P), -1, np.int32)
        tsel = locs_all // P
        for t in range(NT):
            msk = tsel == t
            k = int(msk.sum())
            fi = np.zeros((PO * P,), np.int32)
            fl = np.full((PO * P,), -1, np.int32)
            fi[:k] = rows_all[msk]
            fl[:k] = (locs_all[msk] - t * P)
            oids[t] = fi.reshape(PO, P)
            olocs[t] = fl.reshape(PO, P)
        tl["oids"] = oids.reshape(NT * PO, P).T.copy()
        tl["olocs"] = olocs.reshape(NT * PO, P).T.copy()

        tl.update(shared)
        # pack every resident tile into one tensor per dtype
        npdt = {"b": ml_dtypes.bfloat16, "f": np.float32, "i": np.int32}
        tot = {d: 0 for d in npdt}
        for (_, _, cols, d) in layout:
            tot[d] += cols
        megas = {d: np.zeros((P, tot[d]), npdt[d]) for d in npdt}
        off = {d: 0 for d in npdt}
        for (name, rows, cols, d) in layout:
            arr = np.asarray(tl[name], npdt[d])
            assert arr.shape == (rows, cols), (name, arr.shape, rows, cols)
            megas[d][:rows, off[d]:off[d] + cols] = arr
            off[d] += cols
        m["megab"], m["megaf"], m["megai"] = megas["b"], megas["f"], megas["i"]
        in_maps.append(m)

    return cfg, in_maps


# ---------------------------------------------------------------------------
# Kernel builder
# ---------------------------------------------------------------------------

def build_kernel(cfg):
    OSP, NT, OG = cfg["OSP"], cfg["NT"], cfg["OG"]
    T_PC, NB, S_B = cfg["T_PC"], cfg["NB"], cfg["S_B"]
    PS, PO = cfg["PS"], cfg["PO"]

    nc = bacc.Bacc("TRN2", target_bir_lowering=False, debug=False,
                   num_devices=NC)

    # ---- parameters (all resident tiles packed into 3 mega tensors) ----
    layout = _mega_layout(cfg)
    tot = {"b": 0, "f": 0, "i": 0}
    for (_, _, cols, d) in layout:
        tot[d] += cols
    xt = nc.declare_dram_parameter("xt", [68, OSP], BF16, isOutput=False)
    pred0 = nc.declare_dram_parameter("pred0", [64, T_PC], BF16, isOutput=False)
    megas = {
        "b": nc.declare_dram_parameter("megab", [P, tot["b"]], BF16, isOutput=False),
        "f": nc.declare_dram_parameter("megaf", [P, tot["f"]], F32, isOutput=False),
        "i": nc.declare_dram_parameter("megai", [P, tot["i"]], I32, isOutput=False),
    }
    MDT = {"b": BF16, "f": F32, "i": I32}

    out = nc.declare_dram_parameter("out", [4, OSP], F32, isOutput=True)

    # ---- internal DRAM ----
    tabs = [nc.dram_tensor("tab0", [OG, 64], BF16, addr_space="Shared")]
    for li in range(1, 4):
        tabs.append(nc.dram_tensor(f"tab{li}", [OG, P], BF16, addr_space="Shared"))
    preds = [pred0]
    for li in range(1, 4):
        preds.append(nc.dram_tensor(f"pred{li}", [P, T_PC], BF16))
    stages, sends, recvs, agins = [], [], [], []
    for li, (din, h, dout) in enumerate(DIMS):
        stages.append(nc.dram_tensor(f"stage{li}", [T_PC, h], BF16))
        sends.append(nc.dram_tensor(f"send{li}", [NC * S_B, h], BF16))
        recvs.append(nc.dram_tensor(f"recv{li}", [NC * S_B, h], BF16))
    agins.append(nc.dram_tensor("agin_e", [OSP, 64], BF16))
    for li in range(3):
        agins.append(nc.dram_tensor(f"agin{li}", [OSP, P], BF16))

    PRELU = mybir.ActivationFunctionType.Prelu
    GRPS = [list(range(NC))]

    with tile.TileContext(nc) as tc:
        with tc.tile_pool(name="cst", bufs=1) as cst:
            # constants
            ident = cst.tile([P, P], F32)
            make_identity(nc, ident[:])
            ident_bf = cst.tile([P, P], BF16)
            nc.vector.tensor_copy(out=ident_bf[:], in_=ident[:])
            iota = cst.tile([P, P], I32)
            nc.gpsimd.iota(iota[:], pattern=[[1, P]], base=0, channel_multiplier=0)
            ones_row = cst.tile([1, P], BF16)
            nc.vector.memset(ones_row[:], 1.0)

            W = {}
            moff = {"b": 0, "f": 0, "i": 0}
            for (name, rows, cols, d) in layout:
                t = cst.tile([rows, cols], MDT[d], tag=name)
                nc.sync.dma_start(
                    out=t[:], in_=megas[d][0:rows, moff[d]:moff[d] + cols])
                W[name] = t
                moff[d] += cols

            # ---------------- embedding phase ----------------
            NEB = -(-OSP // 512)
            with (
                tc.tile_pool(name="esb", bufs=3) as esb,
                tc.tile_pool(name="eps", bufs=3, space="PSUM") as eps,
            ):
                for b in range(NEB):
                    c0 = b * 512
                    w = min(512, OSP - c0)
                    xin = esb.tile([68, 512], BF16, tag="xin")
                    nc.sync.dma_start(out=xin[:, :w], in_=xt[:, c0:c0 + w])
                    pse = eps.tile([64, 512], F32, space="PSUM", tag="pse")
                    nc.tensor.matmul(out=pse[:, :w], lhsT=W["w_emb"][:], rhs=xin[:, :w],
                                     start=True, stop=True)
                    ebt = esb.tile([64, 512], BF16, tag="ebt")
                    nc.scalar.activation(out=ebt[:, :w], in_=pse[:, :w], func=PRELU,
                                         bias=W["b_emb"][:, :1], alpha=ALPHA)
                    for q in range(-(-w // P)):
                        qw = min(P, w - q * P)
                        ptr = eps.tile([P, 64], BF16, space="PSUM", tag="ptr")
                        nc.tensor.transpose(out=ptr[:qw, :], in_=ebt[:, q * P:q * P + qw],
                                            identity=ident_bf[:64, :64])
                        ent = esb.tile([P, 64], BF16, tag="ent")
                        nc.vector.tensor_copy(out=ent[:qw, :], in_=ptr[:qw, :])
                        nc.sync.dma_start(out=agins[0][c0 + q * P:c0 + q * P + qw, :],
                                          in_=ent[:qw, :])
            nc.gpsimd.collective_compute(
                "AllGather", mybir.AluOpType.bypass, replica_groups=GRPS,
                ins=[agins[0][:]], outs=[tabs[0][:]])

            # ---------------- layers ----------------
            for li, (din, h, dout) in enumerate(DIMS):
                tab_in = tabs[li]
                pred_in = preds[li]
                stage, send, recv = stages[li], sends[li], recvs[li]
                NH = h // P
                s_cols = (0, h)
                p_cols = (h, h + dout)
                o_cols = (h + dout, 2 * h + dout)

                # ---- phase A: triple MLP ----
                with (
                    tc.tile_pool(name=f"asb{li}", bufs=4) as asb,
                    tc.tile_pool(name=f"apstr{li}", bufs=2, space="PSUM") as aps_tr,
                    tc.tile_pool(name=f"apshid{li}", bufs=3, space="PSUM") as aps_hid,
                    tc.tile_pool(name=f"apsout{li}", bufs=3, space="PSUM") as aps_out,
                ):
                    for j in range(NB):
                        # gather 512 subject rows + 512 object rows
                        # (one [P,1]-offset indirect DMA per 128 rows: multi-
                        # column offset APs are not supported by HW SWDGE)
                        ge = asb.tile([P, 8 * din], BF16, tag="ge")
                        for g in range(8):
                            nc.gpsimd.indirect_dma_start(
                                out=ge[:, g * din:(g + 1) * din], out_offset=None,
                                in_=tab_in[:],
                                in_offset=bass.IndirectOffsetOnAxis(
                                    ap=W["sgog"][:, 8 * j + g:8 * j + g + 1], axis=0))
                        sT = asb.tile([din, 512], BF16, tag="sT")
                        oT = asb.tile([din, 512], BF16, tag="oT")
                        for g in range(8):
                            dst = sT if g < 4 else oT
                            gg = g % 4
                            ptr = aps_tr.tile([din, P], BF16, space="PSUM", tag="ptr")
                            nc.tensor.transpose(out=ptr[:], in_=ge[:, g * din:(g + 1) * din],
                                                identity=ident_bf[:])
                            nc.vector.tensor_copy(out=dst[:, gg * P:(gg + 1) * P],
                                                  in_=ptr[:])
                        pT = asb.tile([din, 512], BF16, tag="pT")
                        nc.sync.dma_start(out=pT[:], in_=pred_in[:din, 512 * j:512 * (j + 1)])

                        # hid
                        hidT = []
                        for mh in range(NH):
                            ph = aps_hid.tile([P, 512], F32, space="PSUM", tag="ph")
                            for ki, src in enumerate((sT, pT, oT)):
                                nc.tensor.matmul(
                                    out=ph[:],
                                    lhsT=W[f"w1a{li}_c{ki}"][:, mh * P:(mh + 1) * P],
                                    rhs=src[:],
                                    start=(ki == 0), stop=(ki == 2))
                            ht = asb.tile([P, 512], BF16, tag=f"hidT{mh}",
                                          name=f"hidT{mh}")
                            nc.scalar.activation(out=ht[:], in_=ph[:], func=PRELU,
                                                 bias=W[f"b1a{li}"][:, mh:mh + 1],
                                                 alpha=ALPHA)
                            hidT.append(ht)

                        # new_s / new_o (entry-major), bias via K=1 matmul,
                        # 1/count scale folded into the activation
                        ovs = asb.tile([P, 4 * h], BF16, tag="ovs")
                        ovo = asb.tile([P, 4 * h], BF16, tag="ovo")
                        for (cols, brow, ivname, dst) in (
                                (s_cols, f"b1bsr{li}", "invs", ovs),
                                (o_cols, f"b1bor{li}", "invo", ovo)):
                            for e in range(4):
                                po = aps_out.tile([P, 512], F32, space="PSUM", tag="po")
                                for k in range(NH):
                                    nc.tensor.matmul(
                                        out=po[:, :h],
                                        lhsT=hidT[k][:, e * P:(e + 1) * P],
                                        rhs=W[f"w1b{li}_{k}"][:, cols[0]:cols[1]],
                                        start=(k == 0), stop=False)
                                nc.tensor.matmul(
                                    out=po[:, :h], lhsT=ones_row[:, :],
                                    rhs=W[brow][:, :],
                                    start=False, stop=True)
                                nc.scalar.activation(
                                    out=dst[:, e * h:(e + 1) * h], in_=po[:, :h],
                                    func=PRELU, alpha=ALPHA,
                                    scale=W[ivname][:, 4 * j + e:4 * j + e + 1])
                        # sequential store of new_s rows
                        nc.sync.dma_start(
                            out=stage[512 * j:512 * (j + 1), :].rearrange(
                                "(e p) h -> p e h", e=4),
                            in_=ovs[:].rearrange("p (e h) -> p e h", e=4))
                        # scatter new_o rows into buckets
                        for e in range(4):
                            nc.gpsimd.indirect_dma_start(
                                out=send[:],
                                out_offset=bass.IndirectOffsetOnAxis(
                                    ap=W["ob"][:, 4 * j + e:4 * j + e + 1], axis=0),
                                in_=ovo[:, e * h:(e + 1) * h], in_offset=None)

                        # new_p (feature-major), not needed after last layer
                        if li < 3:
                            pp = aps_out.tile([P, 512], F32, space="PSUM", tag="po")
                            for k in range(NH):
                                nc.tensor.matmul(
                                    out=pp[:dout, :],
                                    lhsT=W[f"w1b{li}_{k}"][:, p_cols[0]:p_cols[1]],
                                    rhs=hidT[k][:],
                                    start=(k == 0), stop=(k == NH - 1))
                            pv = asb.tile([dout, 512], BF16, tag="pv")
                            nc.scalar.activation(out=pv[:], in_=pp[:dout, :], func=PRELU,
                                                 bias=W[f"b1bp{li}"][:, :1], alpha=ALPHA)
                            nc.sync.dma_start(
                                out=preds[li + 1][:, 512 * j:512 * (j + 1)], in_=pv[:])

                # ---- phase B: AllToAll ----
                nc.gpsimd.collective_compute(
                    "AllToAll", mybir.AluOpType.bypass, replica_groups=GRPS,
                    ins=[send[:]], outs=[recv[:]])

                # ---- phase C: pooling + object MLP ----
                with (
                    tc.tile_pool(name=f"csb{li}", bufs=4) as csb,
                    tc.tile_pool(name=f"cpool{li}", bufs=2, space="PSUM") as cps_pool,
                    tc.tile_pool(name=f"ctr{li}", bufs=2, space="PSUM") as cps_tr,
                    tc.tile_pool(name=f"cmlp{li}", bufs=2, space="PSUM") as cps_mlp,
                ):
                    ng = -(-NT // 4)
                    for grp in range(ng):
                        t0 = grp * 4
                        tn = min(4, NT - t0)
                        gw = tn * P
                        pooledT = [csb.tile([P, 512], BF16, tag=f"pooledT{k}",
                                            name=f"pooledT{k}")
                                   for k in range(NH)]
                        for tt in range(t0, t0 + tn):
                            # gathers: stage rows / recv rows for this tile
                            svals = csb.tile([P, PS * h], BF16, tag="svals")
                            for k in range(PS):
                                nc.gpsimd.indirect_dma_start(
                                    out=svals[:, k * h:(k + 1) * h], out_offset=None,
                                    in_=stage[:],
                                    in_offset=bass.IndirectOffsetOnAxis(
                                        ap=W["sids"][:, PS * tt + k:PS * tt + k + 1], axis=0))
                            ovals = csb.tile([P, PO * h], BF16, tag="ovals")
                            for k in range(PO):
                                nc.gpsimd.indirect_dma_start(
                                    out=ovals[:, k * h:(k + 1) * h], out_offset=None,
                                    in_=recv[:],
                                    in_offset=bass.IndirectOffsetOnAxis(
                                        ap=W["oids"][:, PO * tt + k:PO * tt + k + 1], axis=0))
                            pps = cps_pool.tile([P, 512], F32, space="PSUM", tag="pps")
                            nmm = 0
                            for (vals, PN, locs_t) in ((svals, PS, "slocs"),
                                                       (ovals, PO, "olocs")):
                                for k in range(PN):
                                    oh = csb.tile([P, P], BF16, tag="oh")
                                    nc.vector.tensor_tensor(
                                        out=oh[:],
                                        in0=W[locs_t][:, PN * tt + k:PN * tt + k + 1]
                                            .to_broadcast([P, P]),
                                        in1=iota[:], op=mybir.AluOpType.is_equal)
                                    for mh in range(NH):
                                        # start=True zeroes the whole 2KB
                                        # bank, so only the very first matmul
                                        # into this psum tile may set it
                                        nc.tensor.matmul(
                                            out=pps[:, mh * P:(mh + 1) * P],
                                            lhsT=vals[:, k * h + mh * P:k * h + (mh + 1) * P],
                                            rhs=oh[:],
                                            start=(nmm == 0 and mh == 0),
                                            stop=(nmm == (PS + PO) - 1),
                                            skip_group_check=True)
                                    nmm += 1
                            # pooled tile is already feature-major: psum layout
                            # [feat-chunk, subj] per mh slice
                            for mh in range(NH):
                                nc.vector.tensor_copy(
                                    out=pooledT[mh][:, (tt - t0) * P:(tt - t0 + 1) * P],
                                    in_=pps[:, mh * P:(mh + 1) * P])
                        # object MLP on gw objects
                        hid2 = []
                        for mh in range(NH):
                            p2 = cps_mlp.tile([P, 512], F32, space="PSUM", tag="p2")
                            for k in range(NH):
                                nc.tensor.matmul(
                                    out=p2[:, :gw],
                                    lhsT=W[f"w2a{li}_{k}"][:, mh * P:(mh + 1) * P],
                                    rhs=pooledT[k][:, :gw],
                                    start=(k == 0), stop=(k == NH - 1))
                            h2 = csb.tile([P, 512], BF16, tag=f"h2_{mh}",
                                          name=f"h2_{mh}")
                            nc.scalar.activation(out=h2[:, :gw], in_=p2[:, :gw],
                                                 func=PRELU,
                                                 bias=W[f"b2a{li}"][:, mh:mh + 1],
                                                 alpha=ALPHA)
                            hid2.append(h2)
                        pno = cps_mlp.tile([P, 512], F32, space="PSUM", tag="p2")
                        for k in range(NH):
                            nc.tensor.matmul(out=pno[:dout, :gw],
                                             lhsT=W[f"w2b{li}_{k}"][:],
                                             rhs=hid2[k][:, :gw],
                                             start=(k == 0), stop=(k == NH - 1))
                        noT = csb.tile([dout, 512], BF16, tag="noT")
                        nc.scalar.activation(out=noT[:, :gw], in_=pno[:dout, :gw],
                                             func=PRELU, bias=W[f"b2b{li}"][:, :1],
                                             alpha=ALPHA)
                        if li < 3:
                            for q in range(tn):
                                ptr3 = cps_tr.tile([P, P], BF16, space="PSUM", tag="ptr2")
                                nc.tensor.transpose(out=ptr3[:, :dout],
                                                    in_=noT[:, q * P:(q + 1) * P],
                                                    identity=ident_bf[:])
                                ent2 = csb.tile([P, P], BF16, tag="ent2")
                                nc.vector.tensor_copy(out=ent2[:, :dout],
                                                      in_=ptr3[:, :dout])
                                r0 = (t0 + q) * P
                                nc.sync.dma_start(out=agins[li + 1][r0:r0 + P, :],
                                                  in_=ent2[:, :dout])
                        else:
                            phd = cps_mlp.tile([4, 512], F32, space="PSUM", tag="phd")
                            nc.tensor.matmul(out=phd[:, :gw], lhsT=W["wbb"][:],
                                             rhs=noT[:, :gw], start=True, stop=True)
                            ho = csb.tile([4, 512], F32, tag="ho")
                            nc.scalar.activation(out=ho[:, :gw], in_=phd[:, :gw],
                                                 func=PRELU, bias=W["bbb"][:, :1],
                                                 alpha=ALPHA)
                            nc.sync.dma_start(out=out[:, t0 * P:t0 * P + gw],
                                              in_=ho[:, :gw])

                # ---- phase D: AllGather new object table ----
                if li < 3:
                    nc.gpsimd.collective_compute(
                        "AllGather", mybir.AluOpType.bypass, replica_groups=GRPS,
                        ins=[agins[li + 1][:]], outs=[tabs[li + 1][:]])

    nc.compile()
    return nc


# ---------------------------------------------------------------------------
# Entry point
# ---------------------------------------------------------------------------

_CACHE = {}


def kernel(**inputs) -> np.ndarray:
    cfg, in_maps = preprocess(inputs)
    key = tuple(sorted(cfg.items()))
    if key not in _CACHE:
        _CACHE[key] = build_kernel(cfg)
    nc = _CACHE[key]
    res = run_bass_kernel_spmd(nc, in_maps, list(range(NC)))
    O, OS = cfg["O"], cfg["OS"]
    full = np.zeros((4, O), np.float32)
    for c in range(NC):
        full[:, c * OS:(c + 1) * OS] = res.results[c]["out"][:, :OS]
    return np.ascontiguousarray(full.T)
